# revision 24
# baseline (speedup 1.0000x reference)
"""BallQuery kernel for Trainium2 (Bass/Tile), data-parallel over batch on 8
cores.  v2: group-of-16 masks + multi-engine sign split.

Problem: xyz (8, 16384, 3) points, new_xyz (8, 1024, 3) query centers.
For each query, return the first NSAMPLE=32 point indices (ascending) with
squared distance < RADIUS^2; pad with the first found index; all-sentinel
(N+1) rows when no point is in the ball.  Output int32 (8, 1024, 32).

Algorithm per core (one batch):
  - Points live on PE partitions in chunks of 128; queries on the free axis.
    A K=24 bf16 matmul (3-way bf16 split of coords/norms, exact to ~3e-7)
    computes s = R2 - d2 for a 128-point chunk x all 1024 queries.
  - The s>0 indicator is computed per chunk on a statically assigned engine
    (ACT Sign -> +-1, or Pool/DVE is_gt -> {0,1}), written bf16 [point, query].
  - A second tiny matmul per (chunk, query-tile) with the indicator as the
    stationary operand and bit weights 2^t (t = point mod 16) as the moving
    operand produces, per group of 16 consecutive points, the exact in-ball
    bit pattern mask16 of the group (queries back on partitions).  ACT drains
    psum to a [128 q, 1024 group] u32 plane per m-tile (affine for +-1
    chunks, copy for {0,1} chunks).
  - Pool builds gated keys key = ((1024-j) << 16) | mask16 (0 when empty).
  - DVE top-32 via 4x max8 + 3x match_replace per 256-group slice (u32,
    exact integer sort), merging 4 slices -> the first 32 non-empty groups;
    u16 shift/and arithmetic decodes the embedded masks into 512 candidate
    values bit*(16384 - n); a final max8/match_replace round extracts the
    true first-32 hits; baseline padding semantics finish the row.
"""

import os
import numpy as np

import concourse.bass as bass
import concourse.bacc as bacc
import concourse.mybir as mybir
import concourse.tile as tile
from concourse import bass_utils

F32 = mybir.dt.float32
BF16 = mybir.dt.bfloat16
I32 = mybir.dt.int32
U32 = mybir.dt.uint32
U16 = mybir.dt.uint16

N = 16384  # points per batch
M = 1024  # queries per batch
B = 8  # batches == cores
NS = 32  # samples per query
R2 = 0.15 * 0.15
KD = 24  # distance-matmul contraction rows
G = 13  # points per group (13 mask bits + 11 rank bits = 24-bit keys)
NCH = N // 128  # 128 point chunks
CPC = 11  # group-copy columns per chunk (ceil(140/13)+1 coverage)
NG = NCH * CPC  # 1408 group-copy columns per query
NQ = 4  # column slices per m-tile
QB = [352 * i for i in range(NQ + 1)]  # slice bounds in columns
CB = 8  # chunks per cbatch
NCB = NCH // CB  # 16 cbatches
NMT = M // 128  # 8 query m-tiles
SENT = float(N + 1)

# sign-engine split per cbatch: 'a'=ACT Sign(+-1), 'd'=DVE is_gt({0,1})
# (Pool/GPSIMD cannot read PSUM, so no 'p' here)
SIGN_ENG = os.environ.get("BALLQ_SIGN", "a" * 16)

mul = mybir.AluOpType.mult
add = mybir.AluOpType.add
sub = mybir.AluOpType.subtract
lsr = mybir.AluOpType.logical_shift_right
lsl = mybir.AluOpType.logical_shift_left
band = mybir.AluOpType.bitwise_and
bor = mybir.AluOpType.bitwise_or
isgt = mybir.AluOpType.is_gt
iseq = mybir.AluOpType.is_equal


def build(nc: bass.Bass, repeat: int = 1):
    I16 = mybir.dt.int16
    xs_t = nc.dram_tensor("xs", [KD, N], BF16, kind="ExternalInput")
    qm_t = nc.dram_tensor("qm", [KD, M], BF16, kind="ExternalInput")
    wg_t = nc.dram_tensor("wg", [128, 13 * CPC], BF16, kind="ExternalInput")
    iotaj_t = nc.dram_tensor("iotaj", [128, NG], F32, kind="ExternalInput")
    tpat_t = nc.dram_tensor("tpat", [128, NS * G], U16, kind="ExternalInput")
    out_t = nc.dram_tensor("out", [M, NS], I32, kind="ExternalOutput")
    out_ap = out_t.ap()
    dbg = os.environ.get("BALLQ_DBG", "")
    dbg_t = None
    if dbg:
        dbg_t = nc.dram_tensor("dbg", [128, NMT * NG], U32, kind="ExternalOutput")

    assert len(SIGN_ENG) == NCB

    with tile.TileContext(nc) as tc:
        import contextlib

        with contextlib.ExitStack() as ctx:
            const_pool = ctx.enter_context(tc.tile_pool(name="const", bufs=1))
            h_pool = ctx.enter_context(tc.tile_pool(name="h", bufs=20))
            dps_pool = ctx.enter_context(
                tc.tile_pool(name="dps", bufs=2, space="PSUM")
            )
            mps_pool = ctx.enter_context(
                tc.tile_pool(name="mps", bufs=2, space="PSUM")
            )
            gate_pool = ctx.enter_context(tc.tile_pool(name="gate", bufs=3))
            qv_pool = ctx.enter_context(tc.tile_pool(name="qv", bufs=1))
            cand_pool = ctx.enter_context(tc.tile_pool(name="cand", bufs=2))
            small_pool = ctx.enter_context(tc.tile_pool(name="small", bufs=4))

            # ---------------- constants ----------------
            xs = const_pool.tile([KD, N], BF16)
            for sl in range(8):
                w = N // 8
                nc.sync.dma_start(
                    xs[:, sl * w : (sl + 1) * w], xs_t.ap()[:, sl * w : (sl + 1) * w]
                )
            qm = const_pool.tile([KD, M], BF16)
            nc.sync.dma_start(qm[:], qm_t.ap())
            wg = const_pool.tile([128, 13 * CPC], BF16)
            nc.sync.dma_start(wg[:], wg_t.ap())
            iotaj = const_pool.tile([128, NG], F32)
            nc.sync.dma_start(iotaj[:], iotaj_t.ap())
            tpat = const_pool.tile([128, NS * G], U16)
            nc.sync.dma_start(tpat[:], tpat_t.ap())
            sent = const_pool.tile([128, 1], I32)
            nc.vector.memset(sent[:], SENT)

            # mask plane: [128 q, mt, cc] f32, y = mask13 + corr per column
            maskpl = const_pool.tile([128, NMT * NG], F32)
            mp3 = maskpl[:].rearrange("p (t j) -> p t j", t=NMT)

            for rep in range(repeat):
                hts = [None] * NCH

                def dist_cbatch(cb):
                    eng = SIGN_ENG[cb]
                    for ci in range(CB):
                        c = cb * CB + ci
                        dps = dps_pool.tile([128, M], F32)
                        for half in range(2):
                            nc.tensor.matmul(
                                dps[:, half * 512 : (half + 1) * 512],
                                xs[:, c * 128 : (c + 1) * 128],
                                qm[:, half * 512 : (half + 1) * 512],
                                start=True,
                                stop=True,
                            )
                        h = h_pool.tile([128, M], BF16)
                        if eng == "a":
                            # saturated sigmoid: exact {0,1} step of s>0
                            nc.scalar.activation(
                                h[:],
                                dps[:],
                                mybir.ActivationFunctionType.Sigmoid,
                                scale=1e30,
                            )
                        elif eng == "p":
                            nc.gpsimd.tensor_scalar(
                                h[:], dps[:], 0.0, None, op0=isgt
                            )
                        else:
                            nc.vector.tensor_scalar(
                                h[:], dps[:], 0.0, None, op0=isgt
                            )
                        hts[c] = h

                def mask_cbatch(cb):
                    # mt blocks stride 128 so 11-wide matmuls never cross a
                    # psum bank boundary (88 used cols per block)
                    mps = mps_pool.tile([128, NMT * 128], F32)
                    for mt in range(NMT):
                        for ci in range(CB):
                            c = cb * CB + ci
                            pat = (128 * c) % 13
                            o = mt * 128 + ci * CPC
                            nc.tensor.matmul(
                                mps[:, o : o + CPC],
                                hts[c][:, mt * 128 : (mt + 1) * 128],
                                wg[:, pat * CPC : (pat + 1) * CPC],
                                start=True,
                                stop=True,
                            )
                    # drain psum -> mask plane
                    dst = mp3[:, :, cb * CB * CPC : (cb + 1) * CB * CPC]
                    src = mps[:].rearrange("p (t w) -> p t w", t=NMT)[
                        :, :, 0 : CB * CPC
                    ]
                    nc.scalar.activation(
                        dst, src, mybir.ActivationFunctionType.Copy
                    )

                qvs = [
                    qv_pool.tile([128, NQ * NS], F32, name=f"qv{i}")
                    for i in range(NMT)
                ]

                def quarter(q):
                    lo, hi = QB[q], QB[q + 1]
                    w = hi - lo
                    for mt in range(NMT):
                        msk = mp3[:, mt, lo:hi]
                        gate = gate_pool.tile([128, w], F32)
                        key = gate_pool.tile([128, w], F32)
                        # f32 global keys ({0,1} masks from saturated
                        # sigmoid/is_gt): key = (m>0)*(rank<<13) + m
                        nc.gpsimd.tensor_scalar(gate[:], msk, 0.0, None, op0=isgt)
                        nc.gpsimd.tensor_tensor(
                            gate[:], gate[:], iotaj[:, lo:hi], mul
                        )
                        nc.gpsimd.tensor_tensor(key[:], gate[:], msk, add)
                        qv = qvs[mt]
                        for r in range(4):
                            nc.vector.max(
                                qv[:, q * NS + 8 * r : q * NS + 8 * r + 8], key[:]
                            )
                            if r < 3:
                                nc.vector.match_replace(
                                    out=key[:],
                                    in_to_replace=qv[
                                        :, q * NS + 8 * r : q * NS + 8 * r + 8
                                    ],
                                    in_values=key[:],
                                    imm_value=0.0,
                                )

                qbound = {4 * (i + 1): i for i in range(NQ - 1)}
                dist_cbatch(0)
                for cb in range(1, NCB):
                    mask_cbatch(cb - 1)
                    dist_cbatch(cb)
                    if cb in qbound:
                        quarter(qbound[cb])
                mask_cbatch(NCB - 1)
                quarter(NQ - 1)
                if dbg == "mask":
                    nc.sync.dma_start(dbg_t.ap(), maskpl[:].bitcast(U32))

                # ---------------- per m-tile merge + decode ----------------
                for mt in range(NMT):
                    qv = qvs[mt]
                    vals = small_pool.tile([128, NS], F32)
                    for r in range(4):
                        nc.vector.max(vals[:, 8 * r : 8 * r + 8], qv[:])
                        if r < 3:
                            nc.vector.match_replace(
                                out=qv[:],
                                in_to_replace=vals[:, 8 * r : 8 * r + 8],
                                in_values=qv[:],
                                imm_value=0.0,
                            )
                    # convert the f32 keys to u32 for bit decoding
                    vu = small_pool.tile([128, NS], U32)
                    nc.vector.tensor_scalar(vu[:], vals[:], 1.0, None, op0=mul)
                    if dbg == "vals":
                        nc.sync.dma_start(
                            dbg_t.ap()[:, mt * NS : (mt + 1) * NS], vu[:]
                        )

                    # decode column cc = NG - (vals>>13): chunk c = cc//11,
                    # k = cc%11, group g = (128c - (11c)%13)/13 + k, so
                    # 13g = 117c + 13*((11c)//13 + k).  A = 16397 - 13g;
                    # candidate value = bit * (A - t); idx = 16397 - value.
                    rk = small_pool.tile([128, NS], U32)
                    cc = small_pool.tile([128, NS], U32)
                    ch = small_pool.tile([128, NS], U32)
                    t1 = small_pool.tile([128, NS], U32)
                    t2 = small_pool.tile([128, NS], U32)
                    wk = small_pool.tile([128, NS], U32)
                    A32 = small_pool.tile([128, NS], U32)
                    m32 = small_pool.tile([128, NS], U32)
                    nc.vector.tensor_scalar(rk[:], vu[:], 13, None, op0=lsr)
                    nc.vector.tensor_scalar(
                        cc[:], rk[:], -1.0, float(NG), op0=mul, op1=add
                    )
                    nc.vector.tensor_scalar(ch[:], cc[:], 5958.0, None, op0=mul)
                    nc.vector.tensor_scalar(ch[:], ch[:], 16, None, op0=lsr)
                    nc.vector.tensor_scalar(t1[:], ch[:], 11.0, None, op0=mul)
                    nc.vector.tensor_scalar(t2[:], t1[:], 5042.0, None, op0=mul)
                    nc.vector.tensor_scalar(t2[:], t2[:], 16, None, op0=lsr)
                    nc.vector.tensor_tensor(wk[:], cc[:], t1[:], sub)  # k
                    nc.vector.tensor_tensor(wk[:], wk[:], t2[:], add)
                    nc.vector.tensor_scalar(wk[:], wk[:], 13.0, None, op0=mul)
                    nc.vector.tensor_scalar(
                        A32[:], ch[:], -117.0, 16397.0, op0=mul, op1=add
                    )
                    nc.vector.tensor_tensor(A32[:], A32[:], wk[:], sub)
                    nc.vector.tensor_scalar(m32[:], vu[:], 8191, None, op0=band)

                    cw = NS * G  # 416 candidates
                    Ae = cand_pool.tile([128, cw], U16)
                    mske = cand_pool.tile([128, cw], U16)
                    bits = cand_pool.tile([128, cw], U16)
                    cand = cand_pool.tile([128, cw], U16)
                    ab = (
                        A32[:].rearrange("p (i o) -> p i o", o=1)
                        .to_broadcast([128, NS, G])
                    )
                    mb = (
                        m32[:].rearrange("p (i o) -> p i o", o=1)
                        .to_broadcast([128, NS, G])
                    )
                    a3 = Ae[:].rearrange("p (i t) -> p i t", i=NS)
                    m3 = mske[:].rearrange("p (i t) -> p i t", i=NS)
                    nc.vector.tensor_scalar(a3, ab, 1.0, None, op0=mul)
                    nc.vector.tensor_scalar(m3, mb, 1.0, None, op0=mul)
                    # bits = (msk >> t) & 1 ; cand = bits * (A - t)
                    nc.vector.tensor_tensor(bits[:], mske[:], tpat[:], lsr)
                    nc.vector.tensor_scalar(bits[:], bits[:], 1, None, op0=band)
                    nc.vector.tensor_tensor(cand[:], Ae[:], tpat[:], sub)
                    nc.vector.tensor_tensor(cand[:], cand[:], bits[:], mul)
                    if dbg == "cand":
                        nc.sync.dma_start(
                            dbg_t.ap()[:, mt * cw : (mt + 1) * cw],
                            cand[:].bitcast(U32),
                        )

                    fv = small_pool.tile([128, NS], U16)
                    for r in range(4):
                        nc.vector.max(fv[:, 8 * r : 8 * r + 8], cand[:])
                        if r < 3:
                            nc.vector.match_replace(
                                out=cand[:],
                                in_to_replace=fv[:, 8 * r : 8 * r + 8],
                                in_values=cand[:],
                                imm_value=0.0,
                            )

                    # idx = 16397 - v with padding semantics
                    idxf = small_pool.tile([128, NS], I32)
                    nc.vector.tensor_scalar(
                        idxf[:], fv[:], -1.0, 16397.0, op0=mul, op1=add
                    )
                    inv = small_pool.tile([128, NS], U16)
                    nc.vector.tensor_scalar(inv[:], fv[:], 0.0, None, op0=iseq)
                    nc.vector.copy_predicated(
                        idxf[:], inv[:], idxf[:, 0:1].to_broadcast([128, NS])
                    )
                    nc.vector.copy_predicated(
                        idxf[:],
                        inv[:, 0:1].to_broadcast([128, NS]),
                        sent[:].to_broadcast([128, NS]),
                    )
                    nc.sync.dma_start(
                        out_ap[mt * 128 : (mt + 1) * 128, :], idxf[:]
                    )

    return nc


def _split3(v):
    """3-way bf16 split of float64 array v: v ~ s0 + s1 + s2 (each bf16)."""
    import ml_dtypes

    s0 = v.astype(ml_dtypes.bfloat16)
    r1 = v - s0.astype(np.float64)
    s1 = r1.astype(ml_dtypes.bfloat16)
    r2 = r1 - s1.astype(np.float64)
    s2 = r2.astype(ml_dtypes.bfloat16)
    return s0, s1, s2


def host_prep(xyz_b: np.ndarray, q_b: np.ndarray) -> dict:
    """Per-batch input prep: build xs/qm split tensors + constants."""
    import ml_dtypes

    BF = ml_dtypes.bfloat16
    x = xyz_b.astype(np.float64)  # [N, 3]
    q = q_b.astype(np.float64)  # [M, 3]
    x0, x1, x2 = _split3(x)
    q0, q1, q2 = _split3(q)
    # C_m = R2 - |q|^2 with |q|^2 in f32 to mirror the reference's rounding
    qsq = np.sum(q_b.astype(np.float32) * q_b.astype(np.float32), axis=1)
    C = np.float64(R2) - qsq.astype(np.float64)
    c0, c1, c2 = _split3(C)
    xsq = np.sum(xyz_b.astype(np.float32) * xyz_b.astype(np.float32), axis=1)
    D = -xsq.astype(np.float64)
    d0, d1, d2 = _split3(D)

    ones_n = np.ones(N, BF)
    ones_m = np.ones(M, BF)
    xs_rows = [ones_n, ones_n, ones_n, d0, d1, d2]
    qm_rows = [c0, c1, c2, ones_m, ones_m, ones_m]
    pairs = [(x0, q0), (x0, q1), (x1, q0), (x0, q2), (x2, q0), (x1, q1)]
    for xsplit, qsplit in pairs:
        two_x = (2.0 * xsplit.astype(np.float64)).astype(BF)
        for d in range(3):
            xs_rows.append(two_x[:, d])
            qm_rows.append(qsplit[:, d])
    xs = np.stack(xs_rows, axis=0)  # [24, N] bf16
    qmv = np.stack(qm_rows, axis=0)  # [24, M] bf16

    # wg: 13 phase patterns x 11 columns; chunk c uses pattern (128c)%13.
    # Column k of pattern `pat` holds weight 2^((pat+p)%13) at partition p
    # when (pat+p)//13 == k  (i.e. point 128c+p belongs to group gs_c+k).
    wg = np.zeros((128, 13 * CPC), BF)
    for pat in range(13):
        for p in range(128):
            k = (pat + p) // 13
            t = (pat + p) % 13
            wg[p, pat * CPC + k] = float(2 ** t)

    cc = np.arange(NG, dtype=np.int64)
    iotaj = np.broadcast_to(
        ((NG - cc) << 13).astype(np.float32)[None, :], (128, NG)
    ).copy()
    tpat = np.broadcast_to(
        np.tile(np.arange(G, dtype=np.uint16), NS)[None, :], (128, NS * G)
    ).copy()
    return {"xs": xs, "qm": qmv, "wg": wg, "iotaj": iotaj, "tpat": tpat}


_NC_CACHE = {}
LAST_RESULT = None
TRACE = bool(int(os.environ.get("BALLQ_TRACE", "0")))


def _get_nc(repeat: int = 1):
    if repeat not in _NC_CACHE:
        nc = bacc.Bacc("TRN2", target_bir_lowering=False, debug=False)
        build(nc, repeat)
        nc.compile()
        _NC_CACHE[repeat] = nc
    return _NC_CACHE[repeat]


def kernel(**inputs) -> np.ndarray:
    global LAST_RESULT
    xyz = np.ascontiguousarray(np.asarray(inputs["xyz"], dtype=np.float32))
    new_xyz = np.ascontiguousarray(np.asarray(inputs["new_xyz"], dtype=np.float32))
    assert xyz.shape == (B, N, 3) and new_xyz.shape == (B, M, 3)

    nc = _get_nc(int(os.environ.get("BALLQ_REPEAT", "1")))
    in_maps = [host_prep(xyz[b], new_xyz[b]) for b in range(B)]
    res = bass_utils.run_bass_kernel_spmd(nc, in_maps, list(range(B)), trace=TRACE)
    LAST_RESULT = res
    out = np.stack([res.results[b]["out"] for b in range(B)], axis=0)
    return out.astype(np.int32)


# revision 40
# speedup vs baseline: 1.1087x; 1.1087x over previous
"""BallQuery kernel for Trainium2 (Bass/Tile), data-parallel over batch on 8
cores.  v2: group-of-16 masks + multi-engine sign split.

Problem: xyz (8, 16384, 3) points, new_xyz (8, 1024, 3) query centers.
For each query, return the first NSAMPLE=32 point indices (ascending) with
squared distance < RADIUS^2; pad with the first found index; all-sentinel
(N+1) rows when no point is in the ball.  Output int32 (8, 1024, 32).

Algorithm per core (one batch):
  - Points live on PE partitions in chunks of 128; queries on the free axis.
    A K=24 bf16 matmul (3-way bf16 split of coords/norms, exact to ~3e-7)
    computes s = R2 - d2 for a 128-point chunk x all 1024 queries.
  - The s>0 indicator is computed per chunk on a statically assigned engine
    (ACT Sign -> +-1, or Pool/DVE is_gt -> {0,1}), written bf16 [point, query].
  - A second tiny matmul per (chunk, query-tile) with the indicator as the
    stationary operand and bit weights 2^t (t = point mod 16) as the moving
    operand produces, per group of 16 consecutive points, the exact in-ball
    bit pattern mask16 of the group (queries back on partitions).  ACT drains
    psum to a [128 q, 1024 group] u32 plane per m-tile (affine for +-1
    chunks, copy for {0,1} chunks).
  - Pool builds gated keys key = ((1024-j) << 16) | mask16 (0 when empty).
  - DVE top-32 via 4x max8 + 3x match_replace per 256-group slice (u32,
    exact integer sort), merging 4 slices -> the first 32 non-empty groups;
    u16 shift/and arithmetic decodes the embedded masks into 512 candidate
    values bit*(16384 - n); a final max8/match_replace round extracts the
    true first-32 hits; baseline padding semantics finish the row.
"""

import os
import numpy as np

import concourse.bass as bass
import concourse.bacc as bacc
import concourse.mybir as mybir
import concourse.tile as tile
from concourse import bass_utils

F32 = mybir.dt.float32
BF16 = mybir.dt.bfloat16
I32 = mybir.dt.int32
U32 = mybir.dt.uint32
U16 = mybir.dt.uint16

N = 16384  # points per batch
M = 1024  # queries per batch
B = 8  # batches == cores
NS = 32  # samples per query
R2 = 0.15 * 0.15
KD = 24  # distance-matmul contraction rows
G = 13  # points per group (13 mask bits + 11 rank bits = 24-bit keys)
NCH = N // 128  # 128 point chunks
CPC = 11  # group-copy columns per chunk (ceil(140/13)+1 coverage)
NG = NCH * CPC  # 1408 group-copy columns per query
_qb = os.environ.get("BALLQ_QB", "440,880,1144")
QB = [0] + [int(x) for x in _qb.split(",")] + [NCH * CPC]
NQ = len(QB) - 1  # column slices per m-tile
CB = 8  # chunks per cbatch
NCB = NCH // CB  # 16 cbatches
NMT = M // 128  # 8 query m-tiles
SENT = float(N + 1)

# sign-engine split per cbatch: 'a'=ACT Sign(+-1), 'd'=DVE is_gt({0,1})
# (Pool/GPSIMD cannot read PSUM, so no 'p' here)
SIGN_ENG = os.environ.get("BALLQ_SIGN", "a" * 16)

mul = mybir.AluOpType.mult
add = mybir.AluOpType.add
sub = mybir.AluOpType.subtract
lsr = mybir.AluOpType.logical_shift_right
lsl = mybir.AluOpType.logical_shift_left
band = mybir.AluOpType.bitwise_and
bor = mybir.AluOpType.bitwise_or
isgt = mybir.AluOpType.is_gt
iseq = mybir.AluOpType.is_equal


def build(nc: bass.Bass, repeat: int = 1):
    I16 = mybir.dt.int16
    xs_t = nc.dram_tensor("xs", [KD, N], BF16, kind="ExternalInput")
    qm_t = nc.dram_tensor("qm", [KD, M], BF16, kind="ExternalInput")
    wg_t = nc.dram_tensor("wg", [128, 13 * CPC], BF16, kind="ExternalInput")
    iotaj_t = nc.dram_tensor("iotaj", [128, NG], F32, kind="ExternalInput")
    tpat_t = nc.dram_tensor("tpat", [128, NS * G], U16, kind="ExternalInput")
    out_t = nc.dram_tensor("out", [M, NS], I32, kind="ExternalOutput")
    out_ap = out_t.ap()
    dbg = os.environ.get("BALLQ_DBG", "")
    dbg_t = None
    if dbg:
        dbg_t = nc.dram_tensor("dbg", [128, NMT * NG], U32, kind="ExternalOutput")

    assert len(SIGN_ENG) == NCB

    with tile.TileContext(nc) as tc:
        import contextlib

        with contextlib.ExitStack() as ctx:
            const_pool = ctx.enter_context(tc.tile_pool(name="const", bufs=1))
            h_pool = ctx.enter_context(tc.tile_pool(name="h", bufs=20))
            dps_pool = ctx.enter_context(
                tc.tile_pool(name="dps", bufs=2, space="PSUM")
            )
            mps_pool = ctx.enter_context(
                tc.tile_pool(name="mps", bufs=2, space="PSUM")
            )
            gate_pool = ctx.enter_context(tc.tile_pool(name="gate", bufs=3))
            qv_pool = ctx.enter_context(tc.tile_pool(name="qv", bufs=1))
            cand_pool = ctx.enter_context(tc.tile_pool(name="cand", bufs=3))
            small_pool = ctx.enter_context(tc.tile_pool(name="small", bufs=1))

            # ---------------- constants ----------------
            xs = const_pool.tile([KD, N], BF16)
            for sl in range(8):
                w = N // 8
                nc.sync.dma_start(
                    xs[:, sl * w : (sl + 1) * w], xs_t.ap()[:, sl * w : (sl + 1) * w]
                )
            qm = const_pool.tile([KD, M], BF16)
            nc.sync.dma_start(qm[:], qm_t.ap())
            wg = const_pool.tile([128, 13 * CPC], BF16)
            nc.sync.dma_start(wg[:], wg_t.ap())
            iotaj = const_pool.tile([128, NG], F32)
            nc.sync.dma_start(iotaj[:], iotaj_t.ap())
            tpat = const_pool.tile([128, NS * G], U16)
            nc.sync.dma_start(tpat[:], tpat_t.ap())
            sent = const_pool.tile([128, 1], I32)
            nc.vector.memset(sent[:], SENT)
            bias_i = const_pool.tile([128, 1], F32)
            nc.vector.memset(bias_i[:], 16397.0)

            # mask plane: [128 q, mt, cc] f32, y = mask13 + corr per column
            maskpl = const_pool.tile([128, NMT * NG], F32)
            mp3 = maskpl[:].rearrange("p (t j) -> p t j", t=NMT)

            for rep in range(repeat):
                hts = [None] * NCH

                def dist_cbatch(cb):
                    eng = SIGN_ENG[cb]
                    for ci in range(CB):
                        c = cb * CB + ci
                        dps = dps_pool.tile([128, M], F32)
                        for half in range(2):
                            nc.tensor.matmul(
                                dps[:, half * 512 : (half + 1) * 512],
                                xs[:, c * 128 : (c + 1) * 128],
                                qm[:, half * 512 : (half + 1) * 512],
                                start=True,
                                stop=True,
                            )
                        h = h_pool.tile([128, M], BF16)
                        if eng == "a":
                            # saturated sigmoid: exact {0,1} step of s>0
                            nc.scalar.activation(
                                h[:],
                                dps[:],
                                mybir.ActivationFunctionType.Sigmoid,
                                scale=1e30,
                            )
                        elif eng == "p":
                            nc.gpsimd.tensor_scalar(
                                h[:], dps[:], 0.0, None, op0=isgt
                            )
                        else:
                            nc.vector.tensor_scalar(
                                h[:], dps[:], 0.0, None, op0=isgt
                            )
                        hts[c] = h

                def mask_cbatch(cb):
                    # mt blocks stride 128 so 11-wide matmuls never cross a
                    # psum bank boundary (88 used cols per block)
                    mps = mps_pool.tile([128, NMT * 128], F32)
                    for mt in range(NMT):
                        for ci in range(CB):
                            c = cb * CB + ci
                            pat = (128 * c) % 13
                            o = mt * 128 + ci * CPC
                            nc.tensor.matmul(
                                mps[:, o : o + CPC],
                                hts[c][:, mt * 128 : (mt + 1) * 128],
                                wg[:, pat * CPC : (pat + 1) * CPC],
                                start=True,
                                stop=True,
                            )
                    # drain psum -> mask plane
                    dst = mp3[:, :, cb * CB * CPC : (cb + 1) * CB * CPC]
                    src = mps[:].rearrange("p (t w) -> p t w", t=NMT)[
                        :, :, 0 : CB * CPC
                    ]
                    dthr = int(os.environ.get("BALLQ_DRAIN_DVE", "6"))
                    if cb >= dthr:
                        nc.scalar.activation(
                            dst, src, mybir.ActivationFunctionType.Copy
                        )
                    else:
                        nc.vector.tensor_scalar(dst, src, 1.0, None, op0=mul)

                NXQ = NQ - 2  # slices that get top-32 extraction
                QVW = NXQ * NS + (QB[NQ] - QB[NXQ])
                qvs = [
                    qv_pool.tile([128, QVW], F32, name=f"qv{i}")
                    for i in range(NMT)
                ]

                def quarter(q):
                    lo, hi = QB[q], QB[q + 1]
                    w = hi - lo
                    for mt in range(NMT):
                        msk = mp3[:, mt, lo:hi]
                        qv = qvs[mt]
                        gate = gate_pool.tile([128, w], F32)
                        if q < NXQ:
                            key = gate_pool.tile([128, w], F32)
                        else:
                            # last two slices: gated keys go straight into
                            # the merge buffer, no extraction pass
                            key = qv[:, NXQ * NS + (lo - QB[NXQ]) :
                                     NXQ * NS + (hi - QB[NXQ])]
                        # f32 global keys ({0,1} masks from saturated
                        # sigmoid/is_gt): key = (m>0)*(rank<<13) + m
                        nc.gpsimd.tensor_scalar(gate[:], msk, 0.0, None, op0=isgt)
                        nc.gpsimd.tensor_tensor(
                            gate[:], gate[:], iotaj[:, lo:hi], mul
                        )
                        nc.gpsimd.tensor_tensor(key if q >= NXQ else key[:],
                                                gate[:], msk, add)
                        if q >= NXQ:
                            continue
                        for r in range(4):
                            nc.vector.max(
                                qv[:, q * NS + 8 * r : q * NS + 8 * r + 8], key[:]
                            )
                            if r < 3:
                                nc.vector.match_replace(
                                    out=key[:],
                                    in_to_replace=qv[
                                        :, q * NS + 8 * r : q * NS + 8 * r + 8
                                    ],
                                    in_values=key[:],
                                    imm_value=0.0,
                                )

                qbound = {}
                for qq in range(NQ - 1):
                    cbr = -(-QB[qq + 1] // (CB * CPC))  # ceil
                    if cbr < NCB:
                        qbound[cbr] = qq
                dist_cbatch(0)
                for cb in range(1, NCB):
                    mask_cbatch(cb - 1)
                    dist_cbatch(cb)
                    if cb in qbound:
                        quarter(qbound[cb])
                mask_cbatch(NCB - 1)
                quarter(NQ - 1)
                if dbg == "mask":
                    nc.sync.dma_start(dbg_t.ap(), maskpl[:].bitcast(U32))

                # ---------------- per m-tile merge + decode ----------------
                # phase 1: merges (DVE) -- Pool/ACT decode chains of earlier
                # m-tiles run underneath later merges
                valss, vus = [], []
                for mt in range(NMT):
                    qv = qvs[mt]
                    vals = small_pool.tile([128, NS], F32, name=f"vals{mt}")
                    for r in range(4):
                        nc.vector.max(vals[:, 8 * r : 8 * r + 8], qv[:])
                        if r < 3:
                            nc.vector.match_replace(
                                out=qv[:],
                                in_to_replace=vals[:, 8 * r : 8 * r + 8],
                                in_values=qv[:],
                                imm_value=0.0,
                            )
                    # f32 keys -> u32 on ACT (idle in tail)
                    vu = small_pool.tile([128, NS], U32, name=f"vu{mt}")
                    nc.scalar.activation(
                        vu[:], vals[:], mybir.ActivationFunctionType.Copy
                    )
                    valss.append(vals)
                    vus.append(vu)
                    if dbg == "vals":
                        nc.sync.dma_start(
                            dbg_t.ap()[:, mt * NS : (mt + 1) * NS], vu[:]
                        )

                # phase 2: per-mt decode chains (Pool arithmetic + ACT
                # expansion + DVE bit ops), pipelined across m-tiles
                cands = []
                for mt in range(NMT):
                    vu = vus[mt]
                    # decode column cc = NG - (vu>>13): chunk c = cc//11,
                    # k = cc%11, 13g = 117c + 13*((11c)//13 + k).
                    # A = 16397 - 13g; cand = bit * (A - t); idx = 16397 - v.
                    rk = small_pool.tile([128, NS], U32, name=f"rk{mt}")
                    cc = small_pool.tile([128, NS], U32, name=f"cc{mt}")
                    ch = small_pool.tile([128, NS], U32, name=f"ch{mt}")
                    t1 = small_pool.tile([128, NS], U32, name=f"t1{mt}")
                    t2 = small_pool.tile([128, NS], U32, name=f"t2{mt}")
                    wk = small_pool.tile([128, NS], U32, name=f"wk{mt}")
                    A32 = small_pool.tile([128, NS], U32, name=f"A32{mt}")
                    m32 = small_pool.tile([128, NS], U32, name=f"m32{mt}")
                    nc.vector.tensor_scalar(rk[:], vu[:], 13, None, op0=lsr)
                    nc.gpsimd.tensor_scalar(
                        cc[:], rk[:], -1.0, float(NG), op0=mul, op1=add
                    )
                    nc.gpsimd.tensor_scalar(ch[:], cc[:], 5958.0, None, op0=mul)
                    nc.vector.tensor_scalar(ch[:], ch[:], 16, None, op0=lsr)
                    nc.gpsimd.tensor_scalar(t1[:], ch[:], 11.0, None, op0=mul)
                    nc.gpsimd.tensor_scalar(t2[:], t1[:], 5042.0, None, op0=mul)
                    nc.vector.tensor_scalar(t2[:], t2[:], 16, None, op0=lsr)
                    nc.gpsimd.tensor_tensor(wk[:], cc[:], t1[:], sub)  # k
                    nc.gpsimd.tensor_tensor(wk[:], wk[:], t2[:], add)
                    nc.gpsimd.tensor_scalar(wk[:], wk[:], 13.0, None, op0=mul)
                    nc.gpsimd.tensor_scalar(
                        A32[:], ch[:], -117.0, 16397.0, op0=mul, op1=add
                    )
                    nc.gpsimd.tensor_tensor(A32[:], A32[:], wk[:], sub)
                    nc.vector.tensor_scalar(m32[:], vu[:], 8191, None, op0=band)

                    cw = NS * G  # 416 candidates
                    Ae = cand_pool.tile([128, cw], U16, name="Ae")
                    mske = cand_pool.tile([128, cw], U16, name="mske")
                    bits = cand_pool.tile([128, cw], U16, name="bits")
                    cand = cand_pool.tile([128, cw], U16, name=f"cand{mt}")
                    ab = (
                        A32[:].rearrange("p (i o) -> p i o", o=1)
                        .to_broadcast([128, NS, G])
                    )
                    mb = (
                        m32[:].rearrange("p (i o) -> p i o", o=1)
                        .to_broadcast([128, NS, G])
                    )
                    a3 = Ae[:].rearrange("p (i t) -> p i t", i=NS)
                    m3 = mske[:].rearrange("p (i t) -> p i t", i=NS)
                    nc.scalar.activation(
                        a3, ab, mybir.ActivationFunctionType.Copy
                    )
                    nc.scalar.activation(
                        m3, mb, mybir.ActivationFunctionType.Copy
                    )
                    # bits = (msk >> t) & 1 ; cand = bits * (A - t)
                    nc.vector.tensor_tensor(bits[:], mske[:], tpat[:], lsr)
                    nc.vector.tensor_scalar(bits[:], bits[:], 1, None, op0=band)
                    nc.vector.tensor_tensor(cand[:], Ae[:], tpat[:], sub)
                    nc.vector.tensor_tensor(cand[:], cand[:], bits[:], mul)
                    cands.append(cand)
                    if dbg == "cand":
                        nc.sync.dma_start(
                            dbg_t.ap()[:, mt * cw : (mt + 1) * cw],
                            cand[:].bitcast(U32),
                        )

                # phase 3: final candidate selection + output
                for mt in range(NMT):
                    cand = cands[mt]
                    fv = small_pool.tile([128, NS], U16, name=f"fv{mt}")
                    for r in range(4):
                        nc.vector.max(fv[:, 8 * r : 8 * r + 8], cand[:])
                        if r < 3:
                            nc.vector.match_replace(
                                out=cand[:],
                                in_to_replace=fv[:, 8 * r : 8 * r + 8],
                                in_values=cand[:],
                                imm_value=0.0,
                            )

                    # idx = 16397 - v with padding semantics
                    idxf = small_pool.tile([128, NS], I32, name=f"idxf{mt}")
                    nc.scalar.activation(
                        idxf[:],
                        fv[:],
                        mybir.ActivationFunctionType.Identity,
                        bias=bias_i[:],
                        scale=-1.0,
                    )
                    inv = small_pool.tile([128, NS], U16, name=f"inv{mt}")
                    nc.vector.tensor_scalar(inv[:], fv[:], 0.0, None, op0=iseq)
                    nc.vector.copy_predicated(
                        idxf[:], inv[:], idxf[:, 0:1].to_broadcast([128, NS])
                    )
                    nc.vector.copy_predicated(
                        idxf[:],
                        inv[:, 0:1].to_broadcast([128, NS]),
                        sent[:].to_broadcast([128, NS]),
                    )
                    nc.sync.dma_start(
                        out_ap[mt * 128 : (mt + 1) * 128, :], idxf[:]
                    )

    return nc


def _split3(v):
    """3-way bf16 split of float64 array v: v ~ s0 + s1 + s2 (each bf16)."""
    import ml_dtypes

    s0 = v.astype(ml_dtypes.bfloat16)
    r1 = v - s0.astype(np.float64)
    s1 = r1.astype(ml_dtypes.bfloat16)
    r2 = r1 - s1.astype(np.float64)
    s2 = r2.astype(ml_dtypes.bfloat16)
    return s0, s1, s2


def host_prep(xyz_b: np.ndarray, q_b: np.ndarray) -> dict:
    """Per-batch input prep: build xs/qm split tensors + constants."""
    import ml_dtypes

    BF = ml_dtypes.bfloat16
    x = xyz_b.astype(np.float64)  # [N, 3]
    q = q_b.astype(np.float64)  # [M, 3]
    x0, x1, x2 = _split3(x)
    q0, q1, q2 = _split3(q)
    # C_m = R2 - |q|^2 with |q|^2 in f32 to mirror the reference's rounding
    qsq = np.sum(q_b.astype(np.float32) * q_b.astype(np.float32), axis=1)
    C = np.float64(R2) - qsq.astype(np.float64)
    c0, c1, c2 = _split3(C)
    xsq = np.sum(xyz_b.astype(np.float32) * xyz_b.astype(np.float32), axis=1)
    D = -xsq.astype(np.float64)
    d0, d1, d2 = _split3(D)

    ones_n = np.ones(N, BF)
    ones_m = np.ones(M, BF)
    xs_rows = [ones_n, ones_n, ones_n, d0, d1, d2]
    qm_rows = [c0, c1, c2, ones_m, ones_m, ones_m]
    pairs = [(x0, q0), (x0, q1), (x1, q0), (x0, q2), (x2, q0), (x1, q1)]
    for xsplit, qsplit in pairs:
        two_x = (2.0 * xsplit.astype(np.float64)).astype(BF)
        for d in range(3):
            xs_rows.append(two_x[:, d])
            qm_rows.append(qsplit[:, d])
    xs = np.stack(xs_rows, axis=0)  # [24, N] bf16
    qmv = np.stack(qm_rows, axis=0)  # [24, M] bf16

    # wg: 13 phase patterns x 11 columns; chunk c uses pattern (128c)%13.
    # Column k of pattern `pat` holds weight 2^((pat+p)%13) at partition p
    # when (pat+p)//13 == k  (i.e. point 128c+p belongs to group gs_c+k).
    wg = np.zeros((128, 13 * CPC), BF)
    for pat in range(13):
        for p in range(128):
            k = (pat + p) // 13
            t = (pat + p) % 13
            wg[p, pat * CPC + k] = float(2 ** t)

    cc = np.arange(NG, dtype=np.int64)
    iotaj = np.broadcast_to(
        ((NG - cc) << 13).astype(np.float32)[None, :], (128, NG)
    ).copy()
    tpat = np.broadcast_to(
        np.tile(np.arange(G, dtype=np.uint16), NS)[None, :], (128, NS * G)
    ).copy()
    return {"xs": xs, "qm": qmv, "wg": wg, "iotaj": iotaj, "tpat": tpat}


_NC_CACHE = {}
LAST_RESULT = None
TRACE = bool(int(os.environ.get("BALLQ_TRACE", "0")))


def _get_nc(repeat: int = 1):
    if repeat not in _NC_CACHE:
        nc = bacc.Bacc("TRN2", target_bir_lowering=False, debug=False)
        build(nc, repeat)
        nc.compile()
        _NC_CACHE[repeat] = nc
    return _NC_CACHE[repeat]


def kernel(**inputs) -> np.ndarray:
    global LAST_RESULT
    xyz = np.ascontiguousarray(np.asarray(inputs["xyz"], dtype=np.float32))
    new_xyz = np.ascontiguousarray(np.asarray(inputs["new_xyz"], dtype=np.float32))
    assert xyz.shape == (B, N, 3) and new_xyz.shape == (B, M, 3)

    nc = _get_nc(int(os.environ.get("BALLQ_REPEAT", "1")))
    in_maps = [host_prep(xyz[b], new_xyz[b]) for b in range(B)]
    res = bass_utils.run_bass_kernel_spmd(nc, in_maps, list(range(B)), trace=TRACE)
    LAST_RESULT = res
    out = np.stack([res.results[b]["out"] for b in range(B)], axis=0)
    return out.astype(np.int32)


# revision 45
# speedup vs baseline: 1.1205x; 1.0107x over previous
"""BallQuery kernel for Trainium2 (Bass/Tile), data-parallel over batch on 8
cores.  v2: group-of-16 masks + multi-engine sign split.

Problem: xyz (8, 16384, 3) points, new_xyz (8, 1024, 3) query centers.
For each query, return the first NSAMPLE=32 point indices (ascending) with
squared distance < RADIUS^2; pad with the first found index; all-sentinel
(N+1) rows when no point is in the ball.  Output int32 (8, 1024, 32).

Algorithm per core (one batch):
  - Points live on PE partitions in chunks of 128; queries on the free axis.
    A K=24 bf16 matmul (3-way bf16 split of coords/norms, exact to ~3e-7)
    computes s = R2 - d2 for a 128-point chunk x all 1024 queries.
  - The s>0 indicator is computed per chunk on a statically assigned engine
    (ACT Sign -> +-1, or Pool/DVE is_gt -> {0,1}), written bf16 [point, query].
  - A second tiny matmul per (chunk, query-tile) with the indicator as the
    stationary operand and bit weights 2^t (t = point mod 16) as the moving
    operand produces, per group of 16 consecutive points, the exact in-ball
    bit pattern mask16 of the group (queries back on partitions).  ACT drains
    psum to a [128 q, 1024 group] u32 plane per m-tile (affine for +-1
    chunks, copy for {0,1} chunks).
  - Pool builds gated keys key = ((1024-j) << 16) | mask16 (0 when empty).
  - DVE top-32 via 4x max8 + 3x match_replace per 256-group slice (u32,
    exact integer sort), merging 4 slices -> the first 32 non-empty groups;
    u16 shift/and arithmetic decodes the embedded masks into 512 candidate
    values bit*(16384 - n); a final max8/match_replace round extracts the
    true first-32 hits; baseline padding semantics finish the row.
"""

import os
import numpy as np

import concourse.bass as bass
import concourse.bacc as bacc
import concourse.mybir as mybir
import concourse.tile as tile
from concourse import bass_utils

F32 = mybir.dt.float32
BF16 = mybir.dt.bfloat16
I32 = mybir.dt.int32
U32 = mybir.dt.uint32
U16 = mybir.dt.uint16

N = 16384  # points per batch
M = 1024  # queries per batch
B = 8  # batches == cores
NS = 32  # samples per query
R2 = 0.15 * 0.15
KD = 24  # distance-matmul contraction rows
G = 13  # points per group (13 mask bits + 11 rank bits = 24-bit keys)
NCH = N // 128  # 128 point chunks
CPC = 11  # group-copy columns per chunk (ceil(140/13)+1 coverage)
NG = NCH * CPC  # 1408 group-copy columns per query
_qb = os.environ.get("BALLQ_QB", "528,792,1056,1232")
QB = [0] + [int(x) for x in _qb.split(",")] + [NCH * CPC]
NQ = len(QB) - 1  # column slices per m-tile
CB = 8  # chunks per cbatch
NCB = NCH // CB  # 16 cbatches
NMT = M // 128  # 8 query m-tiles
SENT = float(N + 1)

# sign-engine split per cbatch: 'a'=ACT Sign(+-1), 'd'=DVE is_gt({0,1})
# (Pool/GPSIMD cannot read PSUM, so no 'p' here)
SIGN_ENG = os.environ.get("BALLQ_SIGN", "a" * 16)

mul = mybir.AluOpType.mult
add = mybir.AluOpType.add
sub = mybir.AluOpType.subtract
lsr = mybir.AluOpType.logical_shift_right
lsl = mybir.AluOpType.logical_shift_left
band = mybir.AluOpType.bitwise_and
bor = mybir.AluOpType.bitwise_or
isgt = mybir.AluOpType.is_gt
iseq = mybir.AluOpType.is_equal


def build(nc: bass.Bass, repeat: int = 1):
    I16 = mybir.dt.int16
    xs_t = nc.dram_tensor("xs", [KD, N], BF16, kind="ExternalInput")
    qm_t = nc.dram_tensor("qm", [KD, M], BF16, kind="ExternalInput")
    wg_t = nc.dram_tensor("wg", [128, 13 * CPC], BF16, kind="ExternalInput")
    iotaj_t = nc.dram_tensor("iotaj", [128, NG], F32, kind="ExternalInput")
    tpat_t = nc.dram_tensor("tpat", [128, NS * G], U16, kind="ExternalInput")
    out_t = nc.dram_tensor("out", [M, NS], I32, kind="ExternalOutput")
    out_ap = out_t.ap()
    dbg = os.environ.get("BALLQ_DBG", "")
    dbg_t = None
    if dbg:
        dbg_t = nc.dram_tensor("dbg", [128, NMT * NG], U32, kind="ExternalOutput")

    assert len(SIGN_ENG) == NCB

    with tile.TileContext(nc) as tc:
        import contextlib

        with contextlib.ExitStack() as ctx:
            const_pool = ctx.enter_context(tc.tile_pool(name="const", bufs=1))
            h_pool = ctx.enter_context(tc.tile_pool(name="h", bufs=20))
            dps_pool = ctx.enter_context(
                tc.tile_pool(name="dps", bufs=2, space="PSUM")
            )
            mps_pool = ctx.enter_context(
                tc.tile_pool(name="mps", bufs=2, space="PSUM")
            )
            gate_pool = ctx.enter_context(tc.tile_pool(name="gate", bufs=3))
            qv_pool = ctx.enter_context(tc.tile_pool(name="qv", bufs=1))
            cand_pool = ctx.enter_context(tc.tile_pool(name="cand", bufs=3))
            small_pool = ctx.enter_context(tc.tile_pool(name="small", bufs=1))

            # ---------------- constants ----------------
            xs = const_pool.tile([KD, N], BF16)
            for sl in range(8):
                w = N // 8
                nc.sync.dma_start(
                    xs[:, sl * w : (sl + 1) * w], xs_t.ap()[:, sl * w : (sl + 1) * w]
                )
            qm = const_pool.tile([KD, M], BF16)
            nc.sync.dma_start(qm[:], qm_t.ap())
            wg = const_pool.tile([128, 13 * CPC], BF16)
            nc.sync.dma_start(wg[:], wg_t.ap())
            iotaj = const_pool.tile([128, NG], F32)
            nc.sync.dma_start(iotaj[:], iotaj_t.ap())
            tpat = const_pool.tile([128, NS * G], U16)
            nc.sync.dma_start(tpat[:], tpat_t.ap())
            sent = const_pool.tile([128, 1], I32)
            nc.vector.memset(sent[:], SENT)
            bias_i = const_pool.tile([128, 1], F32)
            nc.vector.memset(bias_i[:], 16397.0)

            # mask plane: [128 q, mt, cc] f32, y = mask13 + corr per column
            maskpl = const_pool.tile([128, NMT * NG], F32)
            mp3 = maskpl[:].rearrange("p (t j) -> p t j", t=NMT)

            for rep in range(repeat):
                hts = [None] * NCH

                def dist_cbatch(cb):
                    eng = SIGN_ENG[cb]
                    for ci in range(CB):
                        c = cb * CB + ci
                        dps = dps_pool.tile([128, M], F32)
                        for half in range(2):
                            nc.tensor.matmul(
                                dps[:, half * 512 : (half + 1) * 512],
                                xs[:, c * 128 : (c + 1) * 128],
                                qm[:, half * 512 : (half + 1) * 512],
                                start=True,
                                stop=True,
                            )
                        h = h_pool.tile([128, M], BF16)
                        if eng == "a":
                            # saturated sigmoid: exact {0,1} step of s>0
                            nc.scalar.activation(
                                h[:],
                                dps[:],
                                mybir.ActivationFunctionType.Sigmoid,
                                scale=1e30,
                            )
                        elif eng == "p":
                            nc.gpsimd.tensor_scalar(
                                h[:], dps[:], 0.0, None, op0=isgt
                            )
                        else:
                            nc.vector.tensor_scalar(
                                h[:], dps[:], 0.0, None, op0=isgt
                            )
                        hts[c] = h

                def mask_cbatch(cb):
                    # mt blocks stride 128 so 11-wide matmuls never cross a
                    # psum bank boundary (88 used cols per block)
                    mps = mps_pool.tile([128, NMT * 128], F32)
                    for mt in range(NMT):
                        for ci in range(CB):
                            c = cb * CB + ci
                            pat = (128 * c) % 13
                            o = mt * 128 + ci * CPC
                            nc.tensor.matmul(
                                mps[:, o : o + CPC],
                                hts[c][:, mt * 128 : (mt + 1) * 128],
                                wg[:, pat * CPC : (pat + 1) * CPC],
                                start=True,
                                stop=True,
                            )
                    # drain psum -> mask plane
                    dst = mp3[:, :, cb * CB * CPC : (cb + 1) * CB * CPC]
                    src = mps[:].rearrange("p (t w) -> p t w", t=NMT)[
                        :, :, 0 : CB * CPC
                    ]
                    dthr = int(os.environ.get("BALLQ_DRAIN_DVE", "6"))
                    if cb >= dthr:
                        nc.scalar.activation(
                            dst, src, mybir.ActivationFunctionType.Copy
                        )
                    else:
                        nc.vector.tensor_scalar(dst, src, 1.0, None, op0=mul)

                NXQ = NQ - 2  # slices that get top-32 extraction
                QVW = NXQ * NS + (QB[NQ] - QB[NXQ])
                qvs = [
                    qv_pool.tile([128, QVW], F32, name=f"qv{i}")
                    for i in range(NMT)
                ]

                def quarter(q):
                    lo, hi = QB[q], QB[q + 1]
                    w = hi - lo
                    for mt in range(NMT):
                        msk = mp3[:, mt, lo:hi]
                        qv = qvs[mt]
                        gate = gate_pool.tile([128, w], F32)
                        if q < NXQ:
                            key = gate_pool.tile([128, w], F32)
                        else:
                            # last two slices: gated keys go straight into
                            # the merge buffer, no extraction pass
                            key = qv[:, NXQ * NS + (lo - QB[NXQ]) :
                                     NXQ * NS + (hi - QB[NXQ])]
                        # f32 global keys ({0,1} masks from saturated
                        # sigmoid/is_gt): key = (m>0)*(rank<<13) + m
                        nc.gpsimd.tensor_scalar(gate[:], msk, 0.0, None, op0=isgt)
                        nc.gpsimd.tensor_tensor(
                            gate[:], gate[:], iotaj[:, lo:hi], mul
                        )
                        nc.gpsimd.tensor_tensor(key if q >= NXQ else key[:],
                                                gate[:], msk, add)
                        if q >= NXQ:
                            continue
                        for r in range(4):
                            nc.vector.max(
                                qv[:, q * NS + 8 * r : q * NS + 8 * r + 8], key[:]
                            )
                            if r < 3:
                                nc.vector.match_replace(
                                    out=key[:],
                                    in_to_replace=qv[
                                        :, q * NS + 8 * r : q * NS + 8 * r + 8
                                    ],
                                    in_values=key[:],
                                    imm_value=0.0,
                                )

                qbound = {}
                for qq in range(NQ - 1):
                    cbr = -(-QB[qq + 1] // (CB * CPC))  # ceil
                    if cbr < NCB:
                        qbound[cbr] = qq
                dist_cbatch(0)
                for cb in range(1, NCB):
                    mask_cbatch(cb - 1)
                    dist_cbatch(cb)
                    if cb in qbound:
                        quarter(qbound[cb])
                mask_cbatch(NCB - 1)
                quarter(NQ - 1)
                if dbg == "mask":
                    nc.sync.dma_start(dbg_t.ap(), maskpl[:].bitcast(U32))

                # ---------------- per m-tile merge + decode ----------------
                # phase 1: merges (DVE) -- Pool/ACT decode chains of earlier
                # m-tiles run underneath later merges
                valss, vus = [], []
                for mt in range(NMT):
                    qv = qvs[mt]
                    vals = small_pool.tile([128, NS], F32, name=f"vals{mt}")
                    for r in range(4):
                        nc.vector.max(vals[:, 8 * r : 8 * r + 8], qv[:])
                        if r < 3:
                            nc.vector.match_replace(
                                out=qv[:],
                                in_to_replace=vals[:, 8 * r : 8 * r + 8],
                                in_values=qv[:],
                                imm_value=0.0,
                            )
                    # f32 keys -> u32 on ACT (idle in tail)
                    vu = small_pool.tile([128, NS], U32, name=f"vu{mt}")
                    nc.scalar.activation(
                        vu[:], vals[:], mybir.ActivationFunctionType.Copy
                    )
                    valss.append(vals)
                    vus.append(vu)
                    if dbg == "vals":
                        nc.sync.dma_start(
                            dbg_t.ap()[:, mt * NS : (mt + 1) * NS], vu[:]
                        )

                def phase3(mt):
                    cand = cands[mt]
                    fv = small_pool.tile([128, NS], U16, name=f"fv{mt}")
                    for r in range(4):
                        nc.vector.max(fv[:, 8 * r : 8 * r + 8], cand[:])
                        if r < 3:
                            nc.vector.match_replace(
                                out=cand[:],
                                in_to_replace=fv[:, 8 * r : 8 * r + 8],
                                in_values=cand[:],
                                imm_value=0.0,
                            )
                    idxf = small_pool.tile([128, NS], I32, name=f"idxf{mt}")
                    nc.scalar.activation(
                        idxf[:],
                        fv[:],
                        mybir.ActivationFunctionType.Identity,
                        bias=bias_i[:],
                        scale=-1.0,
                    )
                    inv = small_pool.tile([128, NS], U16, name=f"inv{mt}")
                    nc.vector.tensor_scalar(inv[:], fv[:], 0.0, None, op0=iseq)
                    nc.vector.copy_predicated(
                        idxf[:], inv[:], idxf[:, 0:1].to_broadcast([128, NS])
                    )
                    nc.vector.copy_predicated(
                        idxf[:],
                        inv[:, 0:1].to_broadcast([128, NS]),
                        sent[:].to_broadcast([128, NS]),
                    )
                    nc.sync.dma_start(
                        out_ap[mt * 128 : (mt + 1) * 128, :], idxf[:]
                    )

                # phase 2: per-mt decode chains (Pool arithmetic + ACT
                # expansion + DVE bit ops), pipelined across m-tiles
                cands = []
                for mt in range(NMT):
                    vu = vus[mt]
                    # decode column cc = NG - (vu>>13): chunk c = cc//11,
                    # k = cc%11, 13g = 117c + 13*((11c)//13 + k).
                    # A = 16397 - 13g; cand = bit * (A - t); idx = 16397 - v.
                    rk = small_pool.tile([128, NS], U32, name=f"rk{mt}")
                    cc = small_pool.tile([128, NS], U32, name=f"cc{mt}")
                    ch = small_pool.tile([128, NS], U32, name=f"ch{mt}")
                    t1 = small_pool.tile([128, NS], U32, name=f"t1{mt}")
                    t2 = small_pool.tile([128, NS], U32, name=f"t2{mt}")
                    wk = small_pool.tile([128, NS], U32, name=f"wk{mt}")
                    A32 = small_pool.tile([128, NS], U32, name=f"A32{mt}")
                    m32 = small_pool.tile([128, NS], U32, name=f"m32{mt}")
                    nc.vector.tensor_scalar(rk[:], vu[:], 13, None, op0=lsr)
                    nc.gpsimd.tensor_scalar(
                        cc[:], rk[:], -1.0, float(NG), op0=mul, op1=add
                    )
                    nc.gpsimd.tensor_scalar(ch[:], cc[:], 5958.0, None, op0=mul)
                    nc.vector.tensor_scalar(ch[:], ch[:], 16, None, op0=lsr)
                    nc.gpsimd.tensor_scalar(t1[:], ch[:], 11.0, None, op0=mul)
                    nc.gpsimd.tensor_scalar(t2[:], t1[:], 5042.0, None, op0=mul)
                    nc.vector.tensor_scalar(t2[:], t2[:], 16, None, op0=lsr)
                    nc.gpsimd.tensor_tensor(wk[:], cc[:], t1[:], sub)  # k
                    nc.gpsimd.tensor_tensor(wk[:], wk[:], t2[:], add)
                    nc.gpsimd.tensor_scalar(wk[:], wk[:], 13.0, None, op0=mul)
                    nc.gpsimd.tensor_scalar(
                        A32[:], ch[:], -117.0, 16397.0, op0=mul, op1=add
                    )
                    nc.gpsimd.tensor_tensor(A32[:], A32[:], wk[:], sub)
                    nc.vector.tensor_scalar(m32[:], vu[:], 8191, None, op0=band)

                    cw = NS * G  # 416 candidates
                    Ae = cand_pool.tile([128, cw], U16, name="Ae")
                    mske = cand_pool.tile([128, cw], U16, name="mske")
                    bits = cand_pool.tile([128, cw], U16, name="bits")
                    cand = cand_pool.tile([128, cw], U16, name=f"cand{mt}")
                    ab = (
                        A32[:].rearrange("p (i o) -> p i o", o=1)
                        .to_broadcast([128, NS, G])
                    )
                    mb = (
                        m32[:].rearrange("p (i o) -> p i o", o=1)
                        .to_broadcast([128, NS, G])
                    )
                    a3 = Ae[:].rearrange("p (i t) -> p i t", i=NS)
                    m3 = mske[:].rearrange("p (i t) -> p i t", i=NS)
                    nc.scalar.activation(
                        a3, ab, mybir.ActivationFunctionType.Copy
                    )
                    nc.scalar.activation(
                        m3, mb, mybir.ActivationFunctionType.Copy
                    )
                    # bits = (msk >> t) & 1 ; cand = bits * (A - t)
                    nc.vector.tensor_tensor(bits[:], mske[:], tpat[:], lsr)
                    nc.vector.tensor_scalar(bits[:], bits[:], 1, None, op0=band)
                    nc.vector.tensor_tensor(cand[:], Ae[:], tpat[:], sub)
                    nc.vector.tensor_tensor(cand[:], cand[:], bits[:], mul)
                    cands.append(cand)
                    if dbg == "cand":
                        nc.sync.dma_start(
                            dbg_t.ap()[:, mt * cw : (mt + 1) * cw],
                            cand[:].bitcast(U32),
                        )
                    if mt >= 1:
                        phase3(mt - 1)
                phase3(NMT - 1)

                if False:
                    cand = cands[0]
                    fv = small_pool.tile([128, NS], U16, name=f"fv{mt}")
                    for r in range(4):
                        nc.vector.max(fv[:, 8 * r : 8 * r + 8], cand[:])
                        if r < 3:
                            nc.vector.match_replace(
                                out=cand[:],
                                in_to_replace=fv[:, 8 * r : 8 * r + 8],
                                in_values=cand[:],
                                imm_value=0.0,
                            )

                    # idx = 16397 - v with padding semantics
                    idxf = small_pool.tile([128, NS], I32, name=f"idxf{mt}")
                    nc.scalar.activation(
                        idxf[:],
                        fv[:],
                        mybir.ActivationFunctionType.Identity,
                        bias=bias_i[:],
                        scale=-1.0,
                    )
                    inv = small_pool.tile([128, NS], U16, name=f"inv{mt}")
                    nc.vector.tensor_scalar(inv[:], fv[:], 0.0, None, op0=iseq)
                    nc.vector.copy_predicated(
                        idxf[:], inv[:], idxf[:, 0:1].to_broadcast([128, NS])
                    )
                    nc.vector.copy_predicated(
                        idxf[:],
                        inv[:, 0:1].to_broadcast([128, NS]),
                        sent[:].to_broadcast([128, NS]),
                    )
                    nc.sync.dma_start(
                        out_ap[mt * 128 : (mt + 1) * 128, :], idxf[:]
                    )

    return nc


def _split3(v):
    """3-way bf16 split of float64 array v: v ~ s0 + s1 + s2 (each bf16)."""
    import ml_dtypes

    s0 = v.astype(ml_dtypes.bfloat16)
    r1 = v - s0.astype(np.float64)
    s1 = r1.astype(ml_dtypes.bfloat16)
    r2 = r1 - s1.astype(np.float64)
    s2 = r2.astype(ml_dtypes.bfloat16)
    return s0, s1, s2


def host_prep(xyz_b: np.ndarray, q_b: np.ndarray) -> dict:
    """Per-batch input prep: build xs/qm split tensors + constants."""
    import ml_dtypes

    BF = ml_dtypes.bfloat16
    x = xyz_b.astype(np.float64)  # [N, 3]
    q = q_b.astype(np.float64)  # [M, 3]
    x0, x1, x2 = _split3(x)
    q0, q1, q2 = _split3(q)
    # C_m = R2 - |q|^2 with |q|^2 in f32 to mirror the reference's rounding
    qsq = np.sum(q_b.astype(np.float32) * q_b.astype(np.float32), axis=1)
    C = np.float64(R2) - qsq.astype(np.float64)
    c0, c1, c2 = _split3(C)
    xsq = np.sum(xyz_b.astype(np.float32) * xyz_b.astype(np.float32), axis=1)
    D = -xsq.astype(np.float64)
    d0, d1, d2 = _split3(D)

    ones_n = np.ones(N, BF)
    ones_m = np.ones(M, BF)
    xs_rows = [ones_n, ones_n, ones_n, d0, d1, d2]
    qm_rows = [c0, c1, c2, ones_m, ones_m, ones_m]
    pairs = [(x0, q0), (x0, q1), (x1, q0), (x0, q2), (x2, q0), (x1, q1)]
    for xsplit, qsplit in pairs:
        two_x = (2.0 * xsplit.astype(np.float64)).astype(BF)
        for d in range(3):
            xs_rows.append(two_x[:, d])
            qm_rows.append(qsplit[:, d])
    xs = np.stack(xs_rows, axis=0)  # [24, N] bf16
    qmv = np.stack(qm_rows, axis=0)  # [24, M] bf16

    # wg: 13 phase patterns x 11 columns; chunk c uses pattern (128c)%13.
    # Column k of pattern `pat` holds weight 2^((pat+p)%13) at partition p
    # when (pat+p)//13 == k  (i.e. point 128c+p belongs to group gs_c+k).
    wg = np.zeros((128, 13 * CPC), BF)
    for pat in range(13):
        for p in range(128):
            k = (pat + p) // 13
            t = (pat + p) % 13
            wg[p, pat * CPC + k] = float(2 ** t)

    cc = np.arange(NG, dtype=np.int64)
    iotaj = np.broadcast_to(
        ((NG - cc) << 13).astype(np.float32)[None, :], (128, NG)
    ).copy()
    tpat = np.broadcast_to(
        np.tile(np.arange(G, dtype=np.uint16), NS)[None, :], (128, NS * G)
    ).copy()
    return {"xs": xs, "qm": qmv, "wg": wg, "iotaj": iotaj, "tpat": tpat}


_NC_CACHE = {}
LAST_RESULT = None
TRACE = bool(int(os.environ.get("BALLQ_TRACE", "0")))


def _get_nc(repeat: int = 1):
    if repeat not in _NC_CACHE:
        nc = bacc.Bacc("TRN2", target_bir_lowering=False, debug=False)
        build(nc, repeat)
        nc.compile()
        _NC_CACHE[repeat] = nc
    return _NC_CACHE[repeat]


def kernel(**inputs) -> np.ndarray:
    global LAST_RESULT
    xyz = np.ascontiguousarray(np.asarray(inputs["xyz"], dtype=np.float32))
    new_xyz = np.ascontiguousarray(np.asarray(inputs["new_xyz"], dtype=np.float32))
    assert xyz.shape == (B, N, 3) and new_xyz.shape == (B, M, 3)

    nc = _get_nc(int(os.environ.get("BALLQ_REPEAT", "1")))
    in_maps = [host_prep(xyz[b], new_xyz[b]) for b in range(B)]
    res = bass_utils.run_bass_kernel_spmd(nc, in_maps, list(range(B)), trace=TRACE)
    LAST_RESULT = res
    out = np.stack([res.results[b]["out"] for b in range(B)], axis=0)
    return out.astype(np.int32)
